# revision 29
# baseline (speedup 1.0000x reference)
"""Discounted cumsum (B,H,S,D)=(8,16,4096,128), gamma per head, scan along S.

Strategy: batch-parallel across 8 NeuronCores (1 batch each, all 16 heads).
HBM traffic is the roofline, so I/O is bf16 (rel-err budget 2e-2; bf16 I/O
lands ~3e-3) and the host pre/post-transposes so every device DMA moves
fully contiguous 8KB partition lines.

Per head, X is laid out transposed [D=128 partitions, S=4096 free] and the
whole recurrence y[:, t] = g*y[:, t-1] + x[:, t] runs as ONE DVE
tensor_tensor_scan instruction (state fp32, gamma fp32 broadcast, data bf16).
No PE, no PSUM, no carry chain. Input ring = SP (sync), output ring = Act
(scalar); mixing directions on one ring halves its throughput.
"""
import sys

sys.path.insert(0, "/opt/trn_rl_repo")
import numpy as np

B, H, S, D = 8, 16, 4096, 128

_CACHE = {}


def _build(repeat=1, mode="full"):
    import contextlib

    import concourse.bacc as bacc
    import concourse.tile as tile
    from concourse import mybir

    f32 = mybir.dt.float32
    bf16 = mybir.dt.bfloat16

    nc = bacc.Bacc("TRN2", target_bir_lowering=False, debug=False)

    x_in = nc.declare_dram_parameter("x", [H, D, S], bf16, isOutput=False)
    g_in = nc.declare_dram_parameter("g", [D, H], f32, isOutput=False)
    y_out = nc.declare_dram_parameter("y", [H, D, S], bf16, isOutput=True)

    with tile.TileContext(nc) as tc:
        with (
            tc.tile_pool(name="const", bufs=1) as const_pool,
            tc.tile_pool(name="xp", bufs=4) as x_pool,
            tc.tile_pool(name="op", bufs=3) as out_pool,
        ):
            gt = const_pool.tile([D, H], f32)
            nc.sync.dma_start(out=gt[:], in_=g_in[:])

            xt = [None] * H
            yt = [None] * H

            def stage_in(h):
                xt[h] = x_pool.tile([D, S], bf16, name=f"xt{h}", tag="xt")
                nc.sync.dma_start(out=xt[h][:], in_=x_in[h])

            def stage_scan(h):
                yt[h] = out_pool.tile([D, S], bf16, name=f"yt{h}", tag="yt")
                if mode != "dmaonly":
                    nc.vector.tensor_tensor_scan(
                        out=yt[h][:],
                        data0=gt[:, h : h + 1].broadcast_to([D, S]),
                        data1=xt[h][:],
                        initial=0.0,
                        op0=mybir.AluOpType.mult,
                        op1=mybir.AluOpType.add,
                    )
                    src = yt[h]
                else:
                    src = xt[h]
                if mode != "scanonly":
                    nc.scalar.dma_start(out=y_out[h], in_=src[:])

            loop = tc.For_i(0, repeat, 1) if repeat > 1 else contextlib.nullcontext()
            with loop:
                for i in range(H + 1):
                    if i < H:
                        stage_in(i)
                    if 0 <= i - 1 < H:
                        stage_scan(i - 1)

    nc.compile()
    return nc


def _prep_inputs(tensor, gamma):
    """Full f32 (B,H,S,D) -> per-core input maps in device layout (bf16)."""
    from ml_dtypes import bfloat16

    g = np.ascontiguousarray(
        np.broadcast_to(np.asarray(gamma, np.float32)[None, :], (D, H))
    )
    xb = np.asarray(tensor).astype(bfloat16)
    in_maps = [
        {
            # (H, S, D) -> (H, D, S), contiguous per head
            "x": np.ascontiguousarray(xb[c].transpose(0, 2, 1)),
            "g": g,
        }
        for c in range(B)
    ]
    return in_maps


def _postprocess(y_cores):
    """Per-core device outputs [H, D, S] bf16 -> full (B,H,S,D) f32."""
    y = np.stack(y_cores, axis=0)  # (B, H, D, S) bf16
    return np.ascontiguousarray(y.transpose(0, 1, 3, 2)).astype(np.float32)


def _fast_callable(nc):
    """Cached jitted shard_map callable (avoids per-call retrace)."""
    import jax
    from jax.experimental.shard_map import shard_map
    from jax.sharding import Mesh, NamedSharding, PartitionSpec
    from concourse import bass2jax, mybir

    bass2jax.install_neuronx_cc_hook()
    partition_name = nc.partition_id_tensor.name if nc.partition_id_tensor else None
    in_names, out_names, out_avals, zero_outs = [], [], [], []
    for alloc in nc.m.functions[0].allocations:
        if not isinstance(alloc, mybir.MemoryLocationSet):
            continue
        name = alloc.memorylocations[0].name
        if alloc.kind == "ExternalInput":
            if name != partition_name:
                in_names.append(name)
        elif alloc.kind == "ExternalOutput":
            shape = tuple(alloc.tensor_shape)
            dtype = mybir.dt.np(alloc.dtype)
            out_avals.append(jax.core.ShapedArray(shape, dtype))
            out_names.append(name)
            zero_outs.append(np.zeros(shape, dtype))
    n_params = len(in_names)
    all_in = list(in_names) + list(out_names)
    if partition_name is not None:
        all_in.append(partition_name)

    def _body(*args):
        operands = list(args)
        if partition_name is not None:
            operands.append(bass2jax.partition_id_tensor())
        return tuple(
            bass2jax._bass_exec_p.bind(
                *operands,
                out_avals=tuple(out_avals),
                in_names=tuple(all_in),
                out_names=tuple(out_names),
                lowering_input_output_aliases=(),
                sim_require_finite=True,
                sim_require_nnan=True,
                nc=nc,
            )
        )

    devices = jax.devices()[:B]
    mesh = Mesh(np.asarray(devices), ("core",))
    specs = (PartitionSpec("core"),)
    f = jax.jit(
        shard_map(
            _body,
            mesh=mesh,
            in_specs=specs * (n_params + len(out_names)),
            out_specs=specs * len(out_names),
            check_rep=False,
        ),
        keep_unused=True,
    )
    sharding = NamedSharding(mesh, PartitionSpec("core"))
    dev_zero = [
        jax.device_put(np.zeros((B * z.shape[0], *z.shape[1:]), z.dtype), sharding)
        for z in zero_outs
    ]
    return f, in_names, out_names, out_avals, sharding, dev_zero


def _run_fast(nc, in_maps):
    import jax

    if "fast" not in _CACHE:
        _CACHE["fast"] = _fast_callable(nc)
    f, in_names, out_names, out_avals, sharding, dev_zero = _CACHE["fast"]
    concat_in = [
        jax.device_put(
            np.concatenate([np.asarray(m[nm]) for m in in_maps], axis=0), sharding
        )
        for nm in in_names
    ]
    outs = f(*concat_in, *dev_zero)
    return [
        {
            nm: np.asarray(outs[i]).reshape(B, *out_avals[i].shape)[c]
            for i, nm in enumerate(out_names)
        }
        for c in range(B)
    ]


def _run(tensor, gamma, trace=False, repeat=1):
    from concourse.bass_utils import run_bass_kernel_spmd

    key = f"nc{repeat}"
    if key not in _CACHE:
        _CACHE[key] = _build(repeat)
    nc = _CACHE[key]

    in_maps = _prep_inputs(tensor, gamma)
    if repeat == 1 and not trace:
        try:
            results = _run_fast(nc, in_maps)
            y = _postprocess([results[c]["y"] for c in range(B)])
            return y, None
        except Exception:
            pass  # fall back to the reference path below
    res = run_bass_kernel_spmd(nc, in_maps, core_ids=list(range(B)), trace=trace)
    y = _postprocess([res.results[c]["y"] for c in range(B)])
    return y, res


def kernel(tensor, gamma):
    try:
        y, _ = _run(tensor, gamma)
    except Exception:
        # transient device/pool errors: clear cached state and retry once
        _CACHE.clear()
        y, _ = _run(tensor, gamma)
    return y
